# revision 6
# baseline (speedup 1.0000x reference)
"""Trainium2 kernel for nn_DPCABlock: distance-pruned cross-attention block.

Strategy (per sharding hint): data-parallel over the fused (B*heads) axis for
the LN/proj/cdist/topk/attention stages -- 8 cores, each owning one batch's
half of the heads (4 heads); conv1x1/norm params replicated.  The final
W_out projection + out-LN + MLP are per-position ops, re-sharded over
(batch, L/2) so each core finishes 1/8 of the positions.

Each core computes, for its (batch b, head-group g):
  - channel-LN of x and context (full batch, redundant across the pair)
  - q/k/v projections restricted to its 4 heads' channels, l2norm q,k
  - L1-cdist of its heads' keys vs the 513 sampled queries, min over samples
  - exact top-513 selection (distances are continuous; top-k by value)
  - sparse attention over the selected keys
  - exchange of head-group outputs within the batch pair, then
    W_out + LN + residual gate + MLP on its L-half.

The per-core program is expressed in jax and compiled by the neuron toolchain
onto each NeuronCore via a shard_map over an 8-device mesh; inputs are
sharded/replicated on host exactly as described, outputs gathered and
reassembled on host.
"""

import numpy as np

B, C, HH, WW = 4, 256, 64, 64
HEADS = 8
DH = C // HEADS
MLP = 1024
EPS = 1e-5
L = HH * WW
TOPK = 1 + L // HEADS  # 513
NCORES = 8
HPC = HEADS // 2  # heads per core (4)


def _chan_ln(x, g, b):
    # x: [C, L]; LN over channel dim
    import jax.numpy as jnp

    mu = jnp.mean(x, axis=0, keepdims=True)
    var = jnp.var(x, axis=0, keepdims=True)
    return (x - mu) * (1.0 / jnp.sqrt(var + EPS)) * g[:, None] + b[:, None]


def _core_fn(xb, cb, ri, cn_g, cn_b, qn_g, qn_b, on_g, on_b,
             Wq_g, Wk_g, Wv_g, W_out, gamma,
             W1, b1, bn1_g, bn1_b, W2, b2, bn2_g, bn2_b, half_sel):
    """Runs on ONE core. xb/cb: [C, L] of this core's batch.
    Wq_g/Wk_g/Wv_g: [HPC*DH, C] rows for this core's heads.
    half_sel: [1] int32, 0 or 1 -- which L-half this core finishes.
    Returns y_half: [C, L//2].
    """
    import jax
    import jax.numpy as jnp
    from jax import lax

    cn = _chan_ln(cb, cn_g, cn_b)
    qn = _chan_ln(xb, qn_g, qn_b)

    k = Wk_g @ cn          # [128, L]
    v = Wv_g @ cn          # [128, L]
    q = Wq_g @ qn          # [128, L]

    def fold(t):  # [HPC*DH, L] -> [HPC, L, DH]
        return t.reshape(HPC, DH, L).transpose(0, 2, 1)

    q, k, v = fold(q), fold(k), fold(v)

    def l2n(t):
        n = jnp.sqrt(jnp.sum(t * t, axis=-1, keepdims=True))
        return t / jnp.maximum(n, 1e-12)

    q = l2n(q)
    k = l2n(k)

    # --- distance-based pruning ---
    # ri: [HPC, TOPK] indices into L; Q_small: [HPC, TOPK, DH]
    Qs = jnp.take_along_axis(q, ri[:, :, None], axis=1)

    # running min of L1 distance over the sampled queries, in chunks of 27
    # (513 = 19*27) so the inner abs-diff-reduce is one big fused op per step
    # instead of 513 tiny sequential ones
    Qc = Qs.reshape(HPC, 19, 27, DH).transpose(1, 0, 2, 3)  # [19, HPC, 27, DH]

    def body(m, qc):  # qc: [HPC, 27, DH]
        d = jnp.sum(jnp.abs(k[:, :, None, :] - qc[:, None, :, :]), axis=-1)
        return jnp.minimum(m, d.min(axis=2)), None       # [HPC, L]

    init = jnp.full_like(k[:, :, 0], jnp.inf)  # [HPC, L], inherits sharding vma
    min_d, _ = lax.scan(body, init, Qc)

    _, idx = lax.top_k(-min_d, TOPK)  # [HPC, TOPK]
    k_sel = jnp.take_along_axis(k, idx[:, :, None], axis=1)
    v_sel = jnp.take_along_axis(v, idx[:, :, None], axis=1)

    scores = jnp.einsum('hqd,hkd->hqk', q, k_sel)
    attn = jax.nn.softmax(scores, axis=-1)
    out = jnp.einsum('hqk,hkd->hqd', attn, v_sel)       # [HPC, L, DH]
    out = out.transpose(0, 2, 1).reshape(HPC * DH, L)   # [128, L]

    # --- exchange within the batch pair ---
    # each core needs the OTHER head-group's output, but only on its own
    # L-half; swap L-halves' worth within the pair via ppermute (1MB each way)
    off = half_sel[0] * (L // 2)
    own_h = lax.dynamic_slice(out, (0, off), (HPC * DH, L // 2))
    # send the partner the slice IT needs (its half = 1 - ours);
    # partner's half offset:
    p_off = (1 - half_sel[0]) * (L // 2)
    send = lax.dynamic_slice(out, (0, p_off), (HPC * DH, L // 2))
    perm = [(0, 1), (1, 0), (2, 3), (3, 2), (4, 5), (5, 4), (6, 7), (7, 6)]
    recv = lax.ppermute(send, 'x', perm)                # partner's heads, my half
    # channel order: core with half_sel==0 is head-group 0 (low channels)
    lo = jnp.where(half_sel[0] == 0, 1, 0)
    out_h = jnp.where(lo == 1,
                      jnp.concatenate([own_h, recv], axis=0),
                      jnp.concatenate([recv, own_h], axis=0))
    x_h = lax.dynamic_slice(xb, (0, off), (C, L // 2))

    op = W_out @ out_h                                   # [C, L/2]
    op = _chan_ln(op, on_g, on_b)
    attn_out = gamma[0] * op + x_h

    h = W1 @ attn_out + b1[:, None]
    h = bn1_g[:, None] * h + bn1_b[:, None]
    h = jnp.maximum(h, 0.0)
    h = W2 @ h + b2[:, None]
    h = bn2_g[:, None] * h + bn2_b[:, None]
    return attn_out + h                                  # [C, L/2]


_COMPILED = {}


def _get_compiled():
    if 'fn' in _COMPILED:
        return _COMPILED['fn']
    import jax
    from jax.sharding import Mesh, PartitionSpec as P
    from jax.experimental.shard_map import shard_map
    from functools import partial

    devs = jax.devices()[:NCORES]
    mesh = Mesh(np.array(devs), ('x',))

    sh = P('x')      # leading dim sharded across cores
    rep = P()        # replicated

    in_specs = (sh, sh, sh,                  # xb, cb, ri (per-core stacked)
                rep, rep, rep, rep, rep, rep,  # ln vectors
                sh, sh, sh,                  # Wq_g, Wk_g, Wv_g (per-core rows)
                rep, rep,                    # W_out, gamma
                rep, rep, rep, rep, rep, rep, rep, rep,  # mlp params
                sh)                          # half_sel

    def wrapped(xb, cb, ri, cn_g, cn_b, qn_g, qn_b, on_g, on_b,
                Wq_g, Wk_g, Wv_g, W_out, gamma,
                W1, b1, bn1_g, bn1_b, W2, b2, bn2_g, bn2_b, half_sel):
        # inside shard_map each arg has its per-core shape (leading 1 dim
        # for sharded args)
        return _core_fn(xb[0], cb[0], ri[0],
                        cn_g, cn_b, qn_g, qn_b, on_g, on_b,
                        Wq_g[0], Wk_g[0], Wv_g[0], W_out, gamma,
                        W1, b1, bn1_g, bn1_b, W2, b2, bn2_g, bn2_b,
                        half_sel[0])[None]

    fn = jax.jit(shard_map(wrapped, mesh=mesh, in_specs=in_specs,
                           out_specs=sh))
    _COMPILED['fn'] = fn
    _COMPILED['mesh'] = mesh
    _COMPILED['in_specs'] = in_specs
    return fn


def prepare_args(inputs):
    """Build the (host) argument tuple fed to the compiled sharded fn."""
    xs = np.asarray(inputs['query_source'], np.float32).reshape(B, C, L)
    cs = np.asarray(inputs['context'], np.float32).reshape(B, C, L)
    ri = np.asarray(inputs['rand_ind']).astype(np.int32).reshape(B, 2, HPC, TOPK)

    W_q = np.asarray(inputs['W_q'], np.float32)
    W_kv = np.asarray(inputs['W_kv'], np.float32)
    Wk_full, Wv_full = W_kv[:C], W_kv[C:]

    xb = np.stack([xs[i // 2] for i in range(NCORES)])
    cb = np.stack([cs[i // 2] for i in range(NCORES)])
    rri = np.stack([ri[i // 2, i % 2] for i in range(NCORES)])
    rows = [slice((i % 2) * HPC * DH, ((i % 2) + 1) * HPC * DH)
            for i in range(NCORES)]
    Wq_g = np.stack([W_q[r] for r in rows])
    Wk_g = np.stack([Wk_full[r] for r in rows])
    Wv_g = np.stack([Wv_full[r] for r in rows])
    half_sel = np.array([[i % 2] for i in range(NCORES)], np.int32)

    f32 = lambda n: np.asarray(inputs[n], np.float32)
    return (xb, cb, rri,
            f32('cn_g'), f32('cn_b'), f32('qn_g'), f32('qn_b'),
            f32('on_g'), f32('on_b'),
            Wq_g, Wk_g, Wv_g, f32('W_out'), f32('gamma'),
            f32('W1'), f32('b1'), f32('bn1_g'), f32('bn1_b'),
            f32('W2'), f32('b2'), f32('bn2_g'), f32('bn2_b'), half_sel)


def kernel(**inputs) -> np.ndarray:
    fn = _get_compiled()
    args = prepare_args(inputs)
    y = np.asarray(fn(*args))  # [8, C, L/2]

    # reassemble: core 2b has batch b's first half, core 2b+1 the second
    out = np.empty((B, C, L), np.float32)
    for i in range(NCORES):
        b_, h_ = i // 2, i % 2
        out[b_, :, h_ * (L // 2):(h_ + 1) * (L // 2)] = y[i]
    return out.reshape(B, C, HH, WW)
